# revision 12
# baseline (speedup 1.0000x reference)
"""Trainium2 Bass kernel: single-head causal self-attention.

Problem: B=4, S=2048, D=1024, f32 IO.
  Q = x@Wq + bq; K = x@Wk + bk; V = x@Wv + bv
  out = softmax(causal(Q K^T / sqrt(D))) @ V

Sharding over 8 NeuronCores: core c -> (batch b = c//2, part p = c%2).
Each batch's 2048 query rows are split into eight 256-row windows
W_0..W_7 (causal k-tile depth of W_m is 2m+2, in 128-key tiles).
p=0 takes windows [0,3,4,7] (depths 2,8,10,16), p=1 takes [1,2,5,6]
(depths 4,6,12,14) -- both sum to 36 so attention work is balanced.
The SPMD graph is identical on every core: it processes 4 "slots" with
fixed k-depths L=[4,8,12,16]; which global window sits in which slot is
pure data placement (host permutes Q rows / output rows, and boundary
causal masks for the last 4 k-tiles of each slot are passed as inputs).

K and V projections are split across the core pair: each core projects
its 1024-key half (from its half of x^T, pre-permuted so own-half is
canonical), then the pair exchanges K^T/V halves with an AllGather over
replica groups [[0,1],[2,3],[4,5],[6,7]] through DRAM bounce buffers.

On-device layout: everything is consumed via x^T [d, s], so the host
passes x^T/xq^T (bf16) directly.  Projections produce K^T,Q^T [d,s] and
V [s,d]; attention computes scores^T [k,q] so that softmax sums reduce
via a ones-matmul and attn@V needs no transposes.  Matmuls in bf16,
softmax/normalization in f32.
"""

import sys

import numpy as np

if "/opt/trn_rl_repo" not in sys.path:
    sys.path.insert(0, "/opt/trn_rl_repo")

import ml_dtypes

D = 1024
S = 2048
B = 4
P = 128
NCORES = 8
WINDOWS = {0: [0, 3, 4, 7], 1: [1, 2, 5, 6]}
LDEPTH = [4, 8, 12, 16]
BF16 = ml_dtypes.bfloat16

_GRAPH = None


def _build_graph():
    import concourse.bass as bass  # noqa: F401
    from concourse import bacc, mybir, tile

    f32 = mybir.dt.float32
    b16 = mybir.dt.bfloat16

    nc = bacc.Bacc(None, target_bir_lowering=False, debug=False, num_devices=NCORES)

    # All big inputs are host-pre-arranged into partition-major SBUF images
    # [128, ...] so every DMA descriptor is one large contiguous chunk per
    # partition (strided patterns serialized the HWDGE sequencer).
    xt_d = nc.declare_dram_parameter("xt", [P, 8 * 1024], b16, False)
    xqt_d = nc.declare_dram_parameter("xqt", [P, 8 * 1024], b16, False)
    wq_d = nc.declare_dram_parameter("wq", [P, 8 * D], b16, False)
    wk_d = nc.declare_dram_parameter("wk", [P, 8 * D], b16, False)
    wv_d = nc.declare_dram_parameter("wv", [P, 8 * D], b16, False)
    bq_d = nc.declare_dram_parameter("bq", [P, 8], f32, False)
    bk_d = nc.declare_dram_parameter("bk", [P, 8], f32, False)
    bvb_d = nc.declare_dram_parameter("bvb", [P, D], f32, False)
    mk_d = nc.declare_dram_parameter("masks", [P, 16 * 256], b16, False)
    out_d = nc.declare_dram_parameter("out", [1024, D], f32, True)

    # K and V exchanged in separate 2MB pair AllGathers: 2MB stays on the
    # fast mesh algorithm (~30us); one combined 4MB falls onto the 2-rank
    # ring path at ~29GB/s (~138us measured).
    # bounce buffers hold partition-major SBUF images [128, 8, 1024]
    ksend = nc.dram_tensor("ksend", [P, 8 * 1024], b16)
    krecv = nc.dram_tensor("krecv", [2, P, 8 * 1024], b16)
    vsend = nc.dram_tensor("vsend", [P, 8 * 1024], b16)
    vrecv = nc.dram_tensor("vrecv", [2, P, 8 * 1024], b16)
    groups = [[0, 1], [2, 3], [4, 5], [6, 7]]

    with tile.TileContext(nc) as tc:
        with (
            tc.tile_pool(name="const", bufs=1) as const,
            tc.tile_pool(name="psA", bufs=2, space="PSUM") as psA,
            tc.tile_pool(name="psB", bufs=4, space="PSUM") as psB,
            tc.tile_pool(name="psS", bufs=2, space="PSUM") as psS,
            tc.tile_pool(name="evict", bufs=6) as evict,
        ):
            xt = const.tile([P, 2, 8, 512], b16, name="xt_s")
            xqt = const.tile([P, 2, 8, 512], b16, name="xqt_s")
            w_sb = {
                n: const.tile([P, 2, 8, 512], b16, name=f"w_{n}_s")
                for n in ("q", "k", "v")
            }
            kT = const.tile([P, 8, S], b16, name="kT_s")
            qT = const.tile([P, 8, 1024], b16, name="qT_s")
            vv = const.tile([P, 16, D], b16, name="v_s")
            bq_s = const.tile([P, 8], f32, name="bq_s")
            bk_s = const.tile([P, 8], f32, name="bk_s")
            bvb_s = const.tile([P, D], f32, name="bvb_s")
            mk_s = const.tile([P, 16, 256], b16, name="mk_s")
            ones_s = const.tile([P, 1], b16, name="ones_s")

            # Input DMAs: contiguous partition-major half-images; ordered so
            # the K projection's first matmuls unblock after ~2MB.
            def load2(dst, dram):
                for c in range(2):
                    nc.sync.dma_start(
                        dst[:, c], dram.ap()[:, c * 4096 : (c + 1) * 4096]
                    )

            nc.sync.dma_start(bk_s[:], bk_d.ap())
            nc.sync.dma_start(w_sb["k"][:, 0], wk_d.ap()[:, 0:4096])
            nc.sync.dma_start(xt[:, 0], xt_d.ap()[:, 0:4096])
            nc.sync.dma_start(xt[:, 1], xt_d.ap()[:, 4096:8192])
            nc.sync.dma_start(w_sb["k"][:, 1], wk_d.ap()[:, 4096:8192])
            nc.sync.dma_start(bvb_s[:], bvb_d.ap())
            load2(w_sb["v"], wv_d)
            nc.sync.dma_start(bq_s[:], bq_d.ap())
            load2(w_sb["q"], wq_d)
            load2(xqt, xqt_d)
            nc.sync.dma_start(
                mk_s[:], mk_d.ap().rearrange("p (n f) -> p n f", f=256)
            )
            nc.any.memset(ones_s[:], 1.0)

            ident = mybir.ActivationFunctionType.Identity

            # K^T projection (own 1024 keys):
            # psum[d_out 128, s 512] = sum_di Wk[di,e].T @ xT[di,s]
            for et in range(8):
                for sw in range(2):
                    ps = psA.tile([P, 512], f32, name="ps_mm")
                    for di in range(8):
                        nc.tensor.matmul(
                            ps[:],
                            w_sb["k"][:, et // 4, di, (et % 4) * P : (et % 4 + 1) * P],
                            xt[:, sw, di, :],
                            start=(di == 0),
                            stop=(di == 7),
                        )
                    ko = evict.tile([P, 512], b16, name="ko")
                    nc.scalar.activation(
                        ko[:],
                        ps[:],
                        ident,
                        bias=bk_s[:, et : et + 1],
                        scale=1.0,
                    )
                    nc.gpsimd.dma_start(
                        ksend.ap()[
                            :, et * 1024 + sw * 512 : et * 1024 + (sw + 1) * 512
                        ],
                        ko[:],
                    )
            nc.gpsimd.collective_compute(
                "AllGather",
                mybir.AluOpType.bypass,
                replica_groups=groups,
                ins=[ksend.ap().opt()],
                outs=[krecv.ap().opt()],
            )
            for g in range(2):
                nc.sync.dma_start(
                    kT[:, :, g * 1024 : (g + 1) * 1024],
                    krecv.ap()[g].rearrange("p (o s) -> p o s", s=1024),
                )

            # V projection (own 1024 keys):
            # psum[s 128, d_out 512] = sum_di xT[di,s].T @ Wv[di,d]
            for st in range(8):
                for dw in range(2):
                    ps = psA.tile([P, 512], f32, name="ps_mm")
                    for di in range(8):
                        nc.tensor.matmul(
                            ps[:],
                            xt[:, st // 4, di, (st % 4) * P : (st % 4 + 1) * P],
                            w_sb["v"][:, dw, di, :],
                            start=(di == 0),
                            stop=(di == 7),
                        )
                    vo = evict.tile([P, 512], b16, name="vo")
                    nc.vector.tensor_tensor(
                        vo[:],
                        ps[:],
                        bvb_s[:, dw * 512 : (dw + 1) * 512],
                        mybir.AluOpType.add,
                    )
                    nc.gpsimd.dma_start(
                        vsend.ap()[
                            :, st * 1024 + dw * 512 : st * 1024 + (dw + 1) * 512
                        ],
                        vo[:],
                    )
            nc.gpsimd.collective_compute(
                "AllGather",
                mybir.AluOpType.bypass,
                replica_groups=groups,
                ins=[vsend.ap().opt()],
                outs=[vrecv.ap().opt()],
            )
            for g in range(2):
                nc.sync.dma_start(
                    vv[:, g * 8 : (g + 1) * 8, :],
                    vrecv.ap()[g].rearrange("p (o d) -> p o d", d=D),
                )

            # Q^T projection (this core's 1024 query rows)
            for et in range(8):
                for sw in range(2):
                    ps = psA.tile([P, 512], f32, name="ps_mm")
                    for di in range(8):
                        nc.tensor.matmul(
                            ps[:],
                            w_sb["q"][:, et // 4, di, (et % 4) * P : (et % 4 + 1) * P],
                            xqt[:, sw, di, :],
                            start=(di == 0),
                            stop=(di == 7),
                        )
                    nc.scalar.activation(
                        qT[:, et, sw * 512 : (sw + 1) * 512],
                        ps[:],
                        ident,
                        bias=bq_s[:, et : et + 1],
                        scale=1.0,
                    )

            # Attention: per slot, scores^T [k 128, q 256] per k-tile,
            # exp (f32 psum -> bf16), boundary mask, then
            # O_unnorm[q,d] += expS^T.T @ V and sums[q] += expS^T.T @ ones.
            inv_sqrt_d = float(1.0 / np.sqrt(D))
            exp_f = mybir.ActivationFunctionType.Exp
            for slot in range(4):
                L = LDEPTH[slot]
                q0 = 256 * slot
                pO = [psB.tile([P, 512], f32, name="psO") for _ in range(4)]
                pSm = [psS.tile([P, 1], f32, name="psSum") for _ in range(2)]
                for kt in range(L):
                    ps = psA.tile([P, 256], f32, name="ps_mm")
                    for di in range(8):
                        nc.tensor.matmul(
                            ps[:, :256],
                            kT[:, di, kt * P : (kt + 1) * P],
                            qT[:, di, q0 : q0 + 256],
                            start=(di == 0),
                            stop=(di == 7),
                        )
                    eS = evict.tile([P, 256], b16, name="eS")
                    nc.scalar.activation(eS[:], ps[:, :256], exp_f, scale=inv_sqrt_d)
                    if kt >= L - 4:
                        nc.vector.tensor_tensor(
                            eS[:],
                            eS[:],
                            mk_s[:, slot * 4 + (kt - (L - 4)), :],
                            mybir.AluOpType.mult,
                        )
                    for jj in range(2):
                        st_op = eS[:, jj * P : (jj + 1) * P]
                        nc.tensor.matmul(
                            pSm[jj][:],
                            st_op,
                            ones_s[:],
                            start=(kt == 0),
                            stop=(kt == L - 1),
                        )
                        for half in range(2):
                            nc.tensor.matmul(
                                pO[jj * 2 + half][:],
                                st_op,
                                vv[:, kt, half * 512 : (half + 1) * 512],
                                start=(kt == 0),
                                stop=(kt == L - 1),
                            )
                for jj in range(2):
                    rec = evict.tile([P, 1], f32, name="recip")
                    nc.vector.reciprocal(rec[:], pSm[jj][:])
                    for half in range(2):
                        o_sb = evict.tile([P, 512], f32, name="o_sb")
                        nc.vector.tensor_scalar_mul(o_sb[:], pO[jj * 2 + half][:], rec[:])
                        nc.sync.dma_start(
                            out_d.ap()[
                                q0 + jj * P : q0 + (jj + 1) * P,
                                half * 512 : (half + 1) * 512,
                            ],
                            o_sb[:],
                        )

    nc.compile()
    return nc


def _get_graph():
    global _GRAPH
    if _GRAPH is None:
        _GRAPH = _build_graph()
    return _GRAPH


def _masks_for(p):
    masks = np.zeros((16, P, 256), dtype=BF16)
    # filled below; returned as partition-major image [P, 16*256]
    k_idx = np.arange(P)[:, None]
    q_idx = np.arange(256)[None, :]
    for slot in range(4):
        L = LDEPTH[slot]
        m = WINDOWS[p][slot]
        for r in range(4):
            kt = L - 4 + r
            valid = (kt * P + k_idx) <= (256 * m + q_idx)
            masks[slot * 4 + r] = valid.astype(BF16)
    return np.ascontiguousarray(masks.transpose(1, 0, 2).reshape(P, 16 * 256))


def _make_in_maps(x, Wq, bq, Wk, bk, Wv, bv):
    x = np.asarray(x, dtype=np.float32)

    def wmajor(w, half_axis):
        # [1024 din, 1024 out] -> [128, 2, 8, 512] half-images; half_axis
        # splits the OUTPUT columns (et/dw halves), contiguous per half.
        w = np.asarray(w, dtype=np.float32).astype(BF16)
        a = w.reshape(8, P, 2, 512)  # [di_o, p, half, 512]
        return np.ascontiguousarray(
            a.transpose(1, 2, 0, 3).reshape(P, 8192)
        )

    wq_b = wmajor(Wq, "out")
    wk_b = wmajor(Wk, "out")
    wv_b = wmajor(Wv, "out")
    bq2 = np.ascontiguousarray(np.asarray(bq, np.float32).reshape(8, P).T)
    bk2 = np.ascontiguousarray(np.asarray(bk, np.float32).reshape(8, P).T)
    bvb = np.ascontiguousarray(np.broadcast_to(np.asarray(bv, np.float32), (P, D)))
    masks_by_p = {p: _masks_for(p) for p in (0, 1)}

    def pmajor_halves(a2d):
        # [1024 d, 1024 s] -> [128, 2, 8, 512] flat: s split into halves,
        # each half a contiguous partition-major image.
        a = a2d.reshape(8, P, 2, 512)  # [d_o, p, s_half, 512]
        return np.ascontiguousarray(
            a.transpose(1, 2, 0, 3).reshape(P, 8192)
        )

    in_maps = []
    for c in range(NCORES):
        b, p = divmod(c, 2)
        xT = x[b].T.astype(BF16)
        xt = pmajor_halves(xT[:, p * 1024 : (p + 1) * 1024])
        qcols = np.concatenate(
            [xT[:, 256 * m : 256 * (m + 1)] for m in WINDOWS[p]], axis=1
        )
        xqt = pmajor_halves(qcols)
        in_maps.append(
            dict(
                xt=xt,
                xqt=xqt,
                wq=wq_b,
                wk=wk_b,
                wv=wv_b,
                bq=bq2,
                bk=bk2,
                bvb=bvb,
                masks=masks_by_p[p],
            )
        )
    return in_maps


def _assemble(results):
    out = np.empty((B, S, D), dtype=np.float32)
    for c in range(NCORES):
        b, p = divmod(c, 2)
        o = results[c]["out"]
        for slot, m in enumerate(WINDOWS[p]):
            out[b, 256 * m : 256 * (m + 1)] = o[256 * slot : 256 * (slot + 1)]
    return out


def _run(in_maps, trace=False, **kwargs):
    from concourse.bass_utils import run_bass_kernel_spmd

    nc = _get_graph()
    return run_bass_kernel_spmd(
        nc, in_maps, core_ids=list(range(NCORES)), trace=trace, **kwargs
    )


def kernel(x, Wq, bq, Wk, bk, Wv, bv):
    in_maps = _make_in_maps(x, Wq, bq, Wk, bk, Wv, bv)
    res = _run(in_maps)
    return _assemble(res.results)


def _install_profile_shim():
    """The agent image's ``antenv`` lacks ``axon_hooks``; recreate it so
    run_bass_kernel_spmd(trace=True) can find the NTFF profile hook, and
    stub out the artifact upload (no bucket access here)."""
    import types

    if "antenv.axon_hooks" not in sys.modules:
        mod = types.ModuleType("antenv.axon_hooks")
        mod._hook = None

        def set_axon_ntff_profile_hook(h):
            mod._hook = h

        def get_axon_ntff_profile_hook():
            return mod._hook

        mod.set_axon_ntff_profile_hook = set_axon_ntff_profile_hook
        mod.get_axon_ntff_profile_hook = get_axon_ntff_profile_hook
        sys.modules["antenv.axon_hooks"] = mod

    if sys.modules["antenv.axon_hooks"]._hook is None:
        from trn_agent_boot.trn_boot import _ntff_profile_via_ctypes

        sys.modules["antenv.axon_hooks"].set_axon_ntff_profile_hook(
            _ntff_profile_via_ctypes("/opt/axon/libaxon_pjrt.so")
        )

    from concourse import bass_utils

    bass_utils.upload_artifacts = lambda tmpdir: f"local:{tmpdir}"


def profile(inputs, **kwargs):
    """Run with tracing; returns (exec_time_ns, BassKernelResults)."""
    _install_profile_shim()
    in_maps = _make_in_maps(**inputs)
    res = _run(in_maps, trace=True, **kwargs)
    return res.exec_time_ns, res


# revision 13
# speedup vs baseline: 1.0018x; 1.0018x over previous
"""Trainium2 Bass kernel: single-head causal self-attention.

Problem: B=4, S=2048, D=1024, f32 IO.
  Q = x@Wq + bq; K = x@Wk + bk; V = x@Wv + bv
  out = softmax(causal(Q K^T / sqrt(D))) @ V

Sharding over 8 NeuronCores: core c -> (batch b = c//2, part p = c%2).
Each batch's 2048 query rows are split into eight 256-row windows
W_0..W_7 (causal depth of W_m is 2m+2 k-tiles of 128).  p=0 takes
windows [0,3,4,7], p=1 takes [1,2,5,6] -- both depth-sum to 36 so
attention work is balanced.  The SPMD graph is identical on every core:
4 window "slots" with fixed half-depths Dj=[2,4,6,8]; which global
window sits in which slot is data placement (host permutes Q rows and
output rows; boundary causal masks are inputs).

K/V are PARITY-SPLIT across the core pair: core p projects K^T/V for
global k-tiles {2j+p} ("own", canonical positions), so each window's
causal range is exactly Dj own + Dj peer tiles on every core (uniform).
Attention consumes own-parity tiles straight from SBUF -- no collective
on that path -- while the pair exchanges halves with two 2MB AllGathers
(small enough to stay on the fast mesh algorithm).  The gathered pair
[member0|member1] is merged as peer = member0 + member1 - own, which is
SPMD-uniform (no per-core addressing anywhere in the graph).

Everything is consumed via x^T [d, s]: host passes x^T/xq^T as bf16
partition-major contiguous images (strided DMA patterns serialize the
HWDGE sequencer).  Projections produce K^T,Q^T [d,s] and V [s,d];
attention computes scores^T [k,q] so softmax sums reduce via a
ones-matmul and attn@V needs no transposes.  Matmuls bf16, softmax and
normalization f32.
"""

import sys

import numpy as np

if "/opt/trn_rl_repo" not in sys.path:
    sys.path.insert(0, "/opt/trn_rl_repo")

import ml_dtypes

D = 1024
S = 2048
B = 4
P = 128
NCORES = 8
WINDOWS = {0: [0, 3, 4, 7], 1: [1, 2, 5, 6]}
DDEPTH = [2, 4, 6, 8]  # per-slot half-depth (own tiles = peer tiles = Dj)
BF16 = ml_dtypes.bfloat16

_GRAPH = None


def _build_graph():
    import concourse.bass as bass  # noqa: F401
    from concourse import bacc, mybir, tile

    f32 = mybir.dt.float32
    b16 = mybir.dt.bfloat16

    nc = bacc.Bacc(None, target_bir_lowering=False, debug=False, num_devices=NCORES)

    # Host-pre-arranged partition-major images (contiguous per partition).
    # xt: x^T columns of OWN parity k-tiles, [128, 2 s-half, 8 d_in, 512]
    # w*: [128, 2 out-half, 8 d_in, 512]
    xt_d = nc.declare_dram_parameter("xt", [P, 8192], b16, False)
    xqt_d = nc.declare_dram_parameter("xqt", [P, 8192], b16, False)
    wq_d = nc.declare_dram_parameter("wq", [P, 8192], b16, False)
    wk_d = nc.declare_dram_parameter("wk", [P, 8192], b16, False)
    wv_d = nc.declare_dram_parameter("wv", [P, 8192], b16, False)
    bq_d = nc.declare_dram_parameter("bq", [P, 8], f32, False)
    bk_d = nc.declare_dram_parameter("bk", [P, 8], f32, False)
    bvb_d = nc.declare_dram_parameter("bvb", [P, D], f32, False)
    mk_d = nc.declare_dram_parameter("masks", [P, 16 * 256], b16, False)
    out_d = nc.declare_dram_parameter("out", [1024, D], f32, True)

    # Bounce buffers: partition-major SBUF images [128, 8, 1024].
    ksend = nc.dram_tensor("ksend", [P, 8 * 1024], b16)
    krecv = nc.dram_tensor("krecv", [2, P, 8 * 1024], b16)
    vsend = nc.dram_tensor("vsend", [P, 8 * 1024], b16)
    vrecv = nc.dram_tensor("vrecv", [2, P, 8 * 1024], b16)
    groups = [[0, 1], [2, 3], [4, 5], [6, 7]]

    with tile.TileContext(nc) as tc:
        with (
            tc.tile_pool(name="const", bufs=1) as const,
            tc.tile_pool(name="psA", bufs=2, space="PSUM") as psA,
            tc.tile_pool(name="psB", bufs=4, space="PSUM") as psB,
            tc.tile_pool(name="psS", bufs=2, space="PSUM") as psS,
            tc.tile_pool(name="evict", bufs=6) as evict,
            tc.tile_pool(name="mtmp", bufs=1) as mtmp,
        ):
            xt = const.tile([P, 2, 8, 512], b16, name="xt_s")
            xqt = const.tile([P, 2, 8, 512], b16, name="xqt_s")
            w_sb = {
                n: const.tile([P, 2, 8, 512], b16, name=f"w_{n}_s")
                for n in ("q", "k", "v")
            }
            kT_own = const.tile([P, 8, 1024], b16, name="kT_own")
            kT_peer = const.tile([P, 8, 1024], b16, name="kT_peer")
            vv_own = const.tile([P, 8, D], b16, name="vv_own")
            vv_peer = const.tile([P, 8, D], b16, name="vv_peer")
            qT = const.tile([P, 8, 1024], b16, name="qT_s")
            bq_s = const.tile([P, 8], f32, name="bq_s")
            bk_s = const.tile([P, 8], f32, name="bk_s")
            bvb_s = const.tile([P, D], f32, name="bvb_s")
            mk_s = const.tile([P, 16, 256], b16, name="mk_s")
            ones_s = const.tile([P, 1], b16, name="ones_s")

            # Input DMAs: ordered so K projection unblocks first.
            nc.sync.dma_start(bk_s[:], bk_d.ap())
            nc.sync.dma_start(w_sb["k"][:, 0], wk_d.ap()[:, 0:4096])
            nc.sync.dma_start(xt[:, 0], xt_d.ap()[:, 0:4096])
            nc.sync.dma_start(xt[:, 1], xt_d.ap()[:, 4096:8192])
            nc.sync.dma_start(w_sb["k"][:, 1], wk_d.ap()[:, 4096:8192])
            nc.sync.dma_start(bvb_s[:], bvb_d.ap())
            for c in range(2):
                nc.sync.dma_start(
                    w_sb["v"][:, c], wv_d.ap()[:, c * 4096 : (c + 1) * 4096]
                )
            nc.sync.dma_start(bq_s[:], bq_d.ap())
            for c in range(2):
                nc.sync.dma_start(
                    w_sb["q"][:, c], wq_d.ap()[:, c * 4096 : (c + 1) * 4096]
                )
            for c in range(2):
                nc.sync.dma_start(xqt[:, c], xqt_d.ap()[:, c * 4096 : (c + 1) * 4096])
            nc.sync.dma_start(mk_s[:], mk_d.ap().rearrange("p (n f) -> p n f", f=256))
            nc.any.memset(ones_s[:], 1.0)

            ident = mybir.ActivationFunctionType.Identity

            # K^T projection (own-parity 1024 keys):
            # psum[d_out 128, s 512] = sum_di Wk[di,e].T @ xT[di,s]
            for et in range(8):
                for sw in range(2):
                    ps = psA.tile([P, 512], f32, name="ps_mm")
                    for di in range(8):
                        nc.tensor.matmul(
                            ps[:],
                            w_sb["k"][:, et // 4, di, (et % 4) * P : (et % 4 + 1) * P],
                            xt[:, sw, di, :],
                            start=(di == 0),
                            stop=(di == 7),
                        )
                    nc.scalar.activation(
                        kT_own[:, et, sw * 512 : (sw + 1) * 512],
                        ps[:],
                        ident,
                        bias=bk_s[:, et : et + 1],
                        scale=1.0,
                    )
                    nc.gpsimd.dma_start(
                        ksend.ap()[
                            :, et * 1024 + sw * 512 : et * 1024 + (sw + 1) * 512
                        ],
                        kT_own[:, et, sw * 512 : (sw + 1) * 512],
                    )
            nc.gpsimd.collective_compute(
                "AllGather",
                mybir.AluOpType.bypass,
                replica_groups=groups,
                ins=[ksend.ap().opt()],
                outs=[krecv.ap().opt()],
            )
            # peer = member0 + member1 - own (SPMD-uniform merge)
            ktmp = mtmp.tile([P, 8, 1024], b16, name="merge_tmp")
            nc.sync.dma_start(
                kT_peer[:], krecv.ap()[0].rearrange("p (o s) -> p o s", s=1024)
            )
            nc.sync.dma_start(
                ktmp[:], krecv.ap()[1].rearrange("p (o s) -> p o s", s=1024)
            )
            nc.any.tensor_tensor(kT_peer[:], kT_peer[:], ktmp[:], mybir.AluOpType.add)
            nc.any.tensor_tensor(
                kT_peer[:], kT_peer[:], kT_own[:], mybir.AluOpType.subtract
            )

            # V projection (own-parity 1024 keys):
            # psum[s 128, d_out 512] = sum_di xT[di,s].T @ Wv[di,d]
            for st in range(8):
                for dw in range(2):
                    ps = psA.tile([P, 512], f32, name="ps_mm")
                    for di in range(8):
                        nc.tensor.matmul(
                            ps[:],
                            xt[:, st // 4, di, (st % 4) * P : (st % 4 + 1) * P],
                            w_sb["v"][:, dw, di, :],
                            start=(di == 0),
                            stop=(di == 7),
                        )
                    vo_slice = vv_own[:, st, dw * 512 : (dw + 1) * 512]
                    nc.vector.tensor_tensor(
                        vo_slice,
                        ps[:],
                        bvb_s[:, dw * 512 : (dw + 1) * 512],
                        mybir.AluOpType.add,
                    )
                    nc.gpsimd.dma_start(
                        vsend.ap()[
                            :, st * 1024 + dw * 512 : st * 1024 + (dw + 1) * 512
                        ],
                        vo_slice,
                    )
            nc.gpsimd.collective_compute(
                "AllGather",
                mybir.AluOpType.bypass,
                replica_groups=groups,
                ins=[vsend.ap().opt()],
                outs=[vrecv.ap().opt()],
            )
            vtmp = mtmp.tile([P, 8, 1024], b16, name="merge_tmp")
            nc.sync.dma_start(
                vv_peer[:], vrecv.ap()[0].rearrange("p (o d) -> p o d", d=D)
            )
            nc.sync.dma_start(vtmp[:], vrecv.ap()[1].rearrange("p (o d) -> p o d", d=D))
            nc.any.tensor_tensor(vv_peer[:], vv_peer[:], vtmp[:], mybir.AluOpType.add)
            nc.any.tensor_tensor(
                vv_peer[:], vv_peer[:], vv_own[:], mybir.AluOpType.subtract
            )

            # Q^T projection (this core's 1024 query rows)
            for et in range(8):
                for sw in range(2):
                    ps = psA.tile([P, 512], f32, name="ps_mm")
                    for di in range(8):
                        nc.tensor.matmul(
                            ps[:],
                            w_sb["q"][:, et // 4, di, (et % 4) * P : (et % 4 + 1) * P],
                            xqt[:, sw, di, :],
                            start=(di == 0),
                            stop=(di == 7),
                        )
                    nc.scalar.activation(
                        qT[:, et, sw * 512 : (sw + 1) * 512],
                        ps[:],
                        ident,
                        bias=bq_s[:, et : et + 1],
                        scale=1.0,
                    )

            # Attention: per slot (deepest first), OWN-parity k-tiles then
            # PEER-parity tiles.  scores^T [k 128, q 256] per tile; exp on
            # ACT (f32 psum -> bf16); boundary masks on the last 2 tiles of
            # each phase; O_unnorm[q,d] += expS^T.T @ V; sums via ones-mm.
            inv_sqrt_d = float(1.0 / np.sqrt(D))
            exp_f = mybir.ActivationFunctionType.Exp
            for slot in (3, 2, 1, 0):
                Dj = DDEPTH[slot]
                q0 = 256 * slot
                pO = [psB.tile([P, 512], f32, name="psO") for _ in range(4)]
                pSm = [psS.tile([P, 1], f32, name="psSum") for _ in range(2)]
                for phase, (kTt, vvt) in enumerate(
                    ((kT_own, vv_own), (kT_peer, vv_peer))
                ):
                    for i in range(Dj):
                        ps = psA.tile([P, 256], f32, name="ps_mm")
                        for di in range(8):
                            nc.tensor.matmul(
                                ps[:, :256],
                                kTt[:, di, i * P : (i + 1) * P],
                                qT[:, di, q0 : q0 + 256],
                                start=(di == 0),
                                stop=(di == 7),
                            )
                        eS = evict.tile([P, 256], b16, name="eS")
                        nc.scalar.activation(
                            eS[:], ps[:, :256], exp_f, scale=inv_sqrt_d
                        )
                        if i >= Dj - 2:
                            nc.vector.tensor_tensor(
                                eS[:],
                                eS[:],
                                mk_s[:, slot * 4 + phase * 2 + (i - (Dj - 2)), :],
                                mybir.AluOpType.mult,
                            )
                        first = phase == 0 and i == 0
                        last = phase == 1 and i == Dj - 1
                        for jj in range(2):
                            st_op = eS[:, jj * P : (jj + 1) * P]
                            nc.tensor.matmul(
                                pSm[jj][:], st_op, ones_s[:], start=first, stop=last
                            )
                            for half in range(2):
                                nc.tensor.matmul(
                                    pO[jj * 2 + half][:],
                                    st_op,
                                    vvt[:, i, half * 512 : (half + 1) * 512],
                                    start=first,
                                    stop=last,
                                )
                for jj in range(2):
                    rec = evict.tile([P, 1], f32, name="recip")
                    nc.vector.reciprocal(rec[:], pSm[jj][:])
                    for half in range(2):
                        o_sb = evict.tile([P, 512], f32, name="o_sb")
                        nc.vector.tensor_scalar_mul(
                            o_sb[:], pO[jj * 2 + half][:], rec[:]
                        )
                        nc.sync.dma_start(
                            out_d.ap()[
                                q0 + jj * P : q0 + (jj + 1) * P,
                                half * 512 : (half + 1) * 512,
                            ],
                            o_sb[:],
                        )

    nc.compile()
    return nc


def _get_graph():
    global _GRAPH
    if _GRAPH is None:
        _GRAPH = _build_graph()
    return _GRAPH


def _masks_for(p):
    """Boundary masks [16 = slot*4 + phase*2 + r, 128, 256] -> image [P, 16*256].

    Slot j hosts window m = WINDOWS[p][j] with half-depth Dj; phase 0 = own
    parity (global tile 2i+p), phase 1 = peer parity (2i+1-p).  Masked
    positions are the last two tiles i in {Dj-2, Dj-1} of each phase.
    """
    masks = np.zeros((16, P, 256), dtype=np.float32)
    k_idx = np.arange(P)[:, None]
    q_idx = np.arange(256)[None, :]
    for slot in range(4):
        Dj = DDEPTH[slot]
        m = WINDOWS[p][slot]
        for phase in range(2):
            par = p if phase == 0 else 1 - p
            for r in range(2):
                i = Dj - 2 + r
                g = 2 * i + par
                valid = (g * P + k_idx) <= (256 * m + q_idx)
                masks[slot * 4 + phase * 2 + r] = valid
    return np.ascontiguousarray(
        masks.astype(BF16).transpose(1, 0, 2).reshape(P, 16 * 256)
    )


def _make_in_maps(x, Wq, bq, Wk, bk, Wv, bv):
    x = np.asarray(x, dtype=np.float32)

    def wmajor(w):
        # [1024 din, 1024 out] -> [128, 2 out-half, 8 din, 512] flat image
        w = np.asarray(w, dtype=np.float32).astype(BF16)
        a = w.reshape(8, P, 2, 512)  # [din_o, p, out_half, 512]
        return np.ascontiguousarray(a.transpose(1, 2, 0, 3).reshape(P, 8192))

    def pmajor_halves(a2d):
        # [1024 d, 1024 s] -> [128, 2 s-half, 8 d_o, 512] flat image
        a = a2d.reshape(8, P, 2, 512)
        return np.ascontiguousarray(a.transpose(1, 2, 0, 3).reshape(P, 8192))

    wq_b = wmajor(Wq)
    wk_b = wmajor(Wk)
    wv_b = wmajor(Wv)
    bq2 = np.ascontiguousarray(np.asarray(bq, np.float32).reshape(8, P).T)
    bk2 = np.ascontiguousarray(np.asarray(bk, np.float32).reshape(8, P).T)
    bvb = np.ascontiguousarray(np.broadcast_to(np.asarray(bv, np.float32), (P, D)))
    masks_by_p = {p: _masks_for(p) for p in (0, 1)}

    in_maps = []
    for c in range(NCORES):
        b, p = divmod(c, 2)
        xT = x[b].T.astype(BF16)
        # own-parity k-tiles: global tiles {2i+p}, i=0..7 (128 cols each)
        own_cols = np.concatenate(
            [xT[:, (2 * i + p) * P : (2 * i + p + 1) * P] for i in range(8)], axis=1
        )
        xt = pmajor_halves(own_cols)
        qcols = np.concatenate(
            [xT[:, 256 * m : 256 * (m + 1)] for m in WINDOWS[p]], axis=1
        )
        xqt = pmajor_halves(qcols)
        in_maps.append(
            dict(
                xt=xt,
                xqt=xqt,
                wq=wq_b,
                wk=wk_b,
                wv=wv_b,
                bq=bq2,
                bk=bk2,
                bvb=bvb,
                masks=masks_by_p[p],
            )
        )
    return in_maps


def _assemble(results):
    out = np.empty((B, S, D), dtype=np.float32)
    for c in range(NCORES):
        b, p = divmod(c, 2)
        o = results[c]["out"]
        for slot, m in enumerate(WINDOWS[p]):
            out[b, 256 * m : 256 * (m + 1)] = o[256 * slot : 256 * (slot + 1)]
    return out


def _run(in_maps, trace=False, **kwargs):
    from concourse.bass_utils import run_bass_kernel_spmd

    nc = _get_graph()
    return run_bass_kernel_spmd(
        nc, in_maps, core_ids=list(range(NCORES)), trace=trace, **kwargs
    )


def kernel(x, Wq, bq, Wk, bk, Wv, bv):
    in_maps = _make_in_maps(x, Wq, bq, Wk, bk, Wv, bv)
    res = _run(in_maps)
    return _assemble(res.results)


def _install_profile_shim():
    """The agent image's ``antenv`` lacks ``axon_hooks``; recreate it so
    run_bass_kernel_spmd(trace=True) can find the NTFF profile hook, and
    stub out the artifact upload (no bucket access here)."""
    import types

    if "antenv.axon_hooks" not in sys.modules:
        mod = types.ModuleType("antenv.axon_hooks")
        mod._hook = None

        def set_axon_ntff_profile_hook(h):
            mod._hook = h

        def get_axon_ntff_profile_hook():
            return mod._hook

        mod.set_axon_ntff_profile_hook = set_axon_ntff_profile_hook
        mod.get_axon_ntff_profile_hook = get_axon_ntff_profile_hook
        sys.modules["antenv.axon_hooks"] = mod

    if sys.modules["antenv.axon_hooks"]._hook is None:
        from trn_agent_boot.trn_boot import _ntff_profile_via_ctypes

        sys.modules["antenv.axon_hooks"].set_axon_ntff_profile_hook(
            _ntff_profile_via_ctypes("/opt/axon/libaxon_pjrt.so")
        )

    from concourse import bass_utils

    bass_utils.upload_artifacts = lambda tmpdir: f"local:{tmpdir}"


def profile(inputs, **kwargs):
    """Run with tracing; returns (exec_time_ns, BassKernelResults)."""
    _install_profile_shim()
    in_maps = _make_in_maps(**inputs)
    res = _run(in_maps, trace=True, **kwargs)
    return res.exec_time_ns, res
